# revision 1
# baseline (speedup 1.0000x reference)
"""Trainium2 Bass kernel for nn_ContrastByClassCalculator (MoCo-style
per-class-queue contrastive loss).

Math (reference):
    l_pos[n]  = q[n] . k[n]                                  # [N, 1]
    l_neg[n,:] = q[n] @ queue[cls_labels[n]]                 # [N, K]
    logits = concat([l_pos, l_neg], 1) / T                   # [N, 1+K]
    loss = mean_n( -log_softmax(logits)[n, 0] )

Sharding: the queue [C=100, D=128, K=2048] dominates memory traffic
(~105 MB), so we shard it over classes across the 8 cores (13 classes
each, with a 1-class overlap window for the 12-class cores). Each core
computes the full loss rows for the samples whose label falls in its
class range, reduces them to a scalar partial sum on device, and the
host adds the 8 partials and divides by N.

Per-core device program (SPMD, identical structure on all 8 cores):
  - 13 class slots, each padded to 32 sample rows, packed 4 per
    128-partition "group" (4 groups: 4+4+4+1 slots).
  - Per slot: DMA the class's queue slab [128, 2048] to SBUF, then 4
    matmuls (N=512) with the slot's packed q vectors [128, 32] as
    stationary -> PSUM group tile rows 32s..32s+31.
  - Per group: row-max on DVE, fused exp+row-sum on ACT (both read
    PSUM directly), combined with the positive logit (computed on DVE
    from packed q/k rows).
  - Tail: one Ln pass, per-row loss, validity mask, ones-vector matmul
    to reduce over partitions -> scalar partial.

QDT selects the matmul datatype for the l_neg GEMMs:
  - "f32"  : exact fp32 (PE runs 2 half-speed passes, 4 cyc/col)
  - "f32r" : fp32 data, single-pass reduced-precision mode (1 cyc/col)
  - "bf16" : queue+q cast to bf16 on host (halves HBM traffic,
             1 cyc/col).  Loss error stays ~1e-5 relative because the
             row-max subtraction cancels in log-softmax and per-row
             errors average out over N=512.
The positive logits and the whole softmax run in fp32 regardless.
"""

import os

import numpy as np

import concourse.bacc as bacc
import concourse.mybir as mybir
import concourse.tile as tile
from concourse import bass_utils

# Problem constants (hardcoded per contract; kernel.py must be self-contained)
N = 512
D = 128
C = 100
K = 2048
T = 0.07
INV_T = float(1.0 / T)

N_CORES = 8
SLOTS = 13           # class slots per core (4 cores own 13 classes, 4 own 12)
M_PAD = 32           # rows per slot (PE col-group granularity)
GROUP_SLOTS = [(0, 4), (4, 8), (8, 12), (12, 13)]
N_GROUPS = len(GROUP_SLOTS)
# slab DMA chunks: one dispatch costs ~0.7us on the serial HWDGE ring, so
# ship slabs in a few large transfers.  First chunk is a single slab so the
# first matmul can start as early as possible.  Group 3's single slab ships
# BEFORE group 2's chunk (and groups are processed 0,1,3,2) so that when
# the last chunk lands, only ONE group's softmax chain remains on the tail.
DMA_CHUNKS = [(0, 1), (1, 4), (12, 13), (4, 8), (8, 12)]
GROUP_ORDER = [0, 1, 3, 2]
FP32 = mybir.dt.float32
BF16 = mybir.dt.bfloat16
# class range end per core: 4 cores x 13 classes + 4 cores x 12 classes
CLASS_ENDS = [13, 26, 39, 52, 64, 76, 88, 100]

# Matmul/shipping dtype for the l_neg GEMMs.  bf16 halves HBM traffic (the
# memory-bound axis of this problem) and costs ~3.4e-5 relative loss error;
# set BASS_QDT=f32 for the exact (but ~1.5x slower) variant.
QDT = os.environ.get("BASS_QDT", "bf16")  # "bf16" | "f32" | "f32r"

# cpack column layout (fp32 columns); the matmul lhsT ("qt") ships as its
# own tensor so it can carry the matmul dtype end-to-end (walrus requires
# fp32r/bf16 operands to be typed at the producer, not bitcast at use).
QR_OFF = 0                            # [128, 512]  q rows, group-major
KR_OFF = QR_OFF + N_GROUPS * D        # [128, 512]  k rows, group-major
MSK_OFF = KR_OFF + N_GROUPS * D       # [128, 4]    row validity per group
ONE_OFF = MSK_OFF + N_GROUPS          # [128, 1]    all-ones column
CPACK_W = ONE_OFF + 1

# Results of the last hardware run (for test harnesses): BassKernelResults
last_run = None


def _build_nc():
    """Build the single-core SPMD Bass/Tile program.

    Bacc (not raw Bass): its finalize runs generate_event_semaphores,
    which splits multi-semaphore waits to satisfy the TRN2 1-wait-per-
    instruction constraint walrus enforces.
    """
    nc = bacc.Bacc("TRN2")

    mm_dt = {"f32": FP32, "f32r": mybir.dt.float32r, "bf16": BF16}[QDT]

    cpack_h = nc.dram_tensor("cpack", [D, CPACK_W], FP32, kind="ExternalInput")
    qt_h = nc.dram_tensor("qt", [D, SLOTS * M_PAD], mm_dt, kind="ExternalInput")
    slabs_h = nc.dram_tensor("slabs", [SLOTS, D, K], mm_dt, kind="ExternalInput")
    out_h = nc.dram_tensor("out", [1, 1], FP32, kind="ExternalOutput")

    AX = mybir.AxisListType
    AF = mybir.ActivationFunctionType

    with tile.TileContext(nc) as tc:
        with (
            tc.tile_pool(name="consts", bufs=1) as consts,
            tc.tile_pool(name="small", bufs=1) as small,
            tc.tile_pool(name="scr", bufs=2) as scr,
            tc.tile_pool(name="slab", bufs=1) as slab_pool,
            tc.tile_pool(name="esc", bufs=2) as esc_pool,
            tc.tile_pool(name="psum", bufs=2, space="PSUM") as psum_pool,
        ):
            # DMA dispatch order matters (FIFO per HWDGE ring): first slab
            # chunk, then the small qt, then cpack, then remaining chunks
            # alternating across the two rings.
            slab_tiles = {}  # slot -> (tile, col offset)
            for ci, (c0, c1) in enumerate(DMA_CHUNKS):
                st = slab_pool.tile([D, (c1 - c0) * K], mm_dt, tag=f"slab{c0}")
                nc.sync.dma_start(
                    out=st[:], in_=slabs_h[c0:c1].rearrange("n p k -> p n k")
                )
                for t in range(c0, c1):
                    slab_tiles[t] = (st, (t - c0) * K)
                if c0 == 0:
                    qt = consts.tile([D, SLOTS * M_PAD], mm_dt)
                    nc.sync.dma_start(out=qt[:], in_=qt_h[:])
                    # cpack rides early: the positive logits it carries gate
                    # each group's exp bias, and through that the PSUM slot
                    # releases — shipping it late cascades ~5us down the
                    # whole softmax pipeline.
                    cp = consts.tile([D, CPACK_W], FP32)
                    nc.sync.dma_start(out=cp[:], in_=cpack_h[:])

            # Warm the Exp spline table while the first DMAs stream.
            warm = small.tile([1, 1], FP32)
            nc.vector.memset(warm[:], 0.0)
            nc.scalar.activation(out=warm[:], in_=warm[:], func=AF.Exp)

            # Per-row stats, one column per group. Rows beyond a group's
            # active partitions keep the memset values, which yield a row
            # loss of exactly 0 (and are masked anyway).
            lpos = small.tile([128, N_GROUPS], FP32)
            nc.vector.memset(lpos[:], 0.0)
            nbias = small.tile([128, N_GROUPS], FP32)
            nc.vector.memset(nbias[:], 0.0)
            sneg = small.tile([128, N_GROUPS], FP32)
            nc.vector.memset(sneg[:], 0.0)

            for g in GROUP_ORDER:
                t0, t1 = GROUP_SLOTS[g]
                pg = 32 * (t1 - t0)
                col = slice(g, g + 1)

                # positive logit: per-row q.k (multiply then row-reduce)
                ttr = scr.tile([128, D], FP32, tag="ttr")
                nc.vector.tensor_mul(
                    ttr[0:pg],
                    cp[0:pg, QR_OFF + g * D:QR_OFF + (g + 1) * D],
                    cp[0:pg, KR_OFF + g * D:KR_OFF + (g + 1) * D],
                )
                nc.vector.reduce_sum(
                    out=lpos[0:pg, col], in_=ttr[0:pg], axis=AX.X
                )

                gps = psum_pool.tile([128, K], FP32, tag="gps")
                for s in range(t1 - t0):
                    t = t0 + s
                    st, coff = slab_tiles[t]
                    for j in range(K // 512):
                        nc.tensor.matmul(
                            out=gps[32 * s:32 * s + 32, 512 * j:512 * (j + 1)],
                            lhsT=qt[:, M_PAD * t:M_PAD * (t + 1)],
                            rhs=st[:, coff + 512 * j:coff + 512 * (j + 1)],
                            start=True,
                            stop=True,
                            tile_position=(0, 32 * s),
                        )

                # row max over negatives; fold in the positive logit and the
                # -1/T exp-bias scale: nbias = -max(nm,lpos)/T.  The tiny
                # fold runs on the otherwise-idle GpSimd engine so it cannot
                # queue behind another group's 2.3us reduce on DVE (that
                # delay lands directly on the exp critical path at the tail).
                nm = scr.tile([128, 1], FP32, tag="nm")
                nc.vector.reduce_max(out=nm[0:pg], in_=gps[0:pg], axis=AX.X)
                nc.gpsimd.tensor_scalar(
                    out=nbias[0:pg, col],
                    in0=nm[0:pg],
                    scalar1=lpos[0:pg, col],
                    scalar2=-INV_T,
                    op0=mybir.AluOpType.max,
                    op1=mybir.AluOpType.mult,
                )

                # exp((l - rmax)/T) with fused row-sum on ACT
                esc = esc_pool.tile([128, K], FP32, tag="esc")
                nc.scalar.activation(
                    out=esc[0:pg],
                    in_=gps[0:pg],
                    func=AF.Exp,
                    bias=nbias[0:pg, col],
                    scale=INV_T,
                    accum_out=sneg[0:pg, col],
                )

            # Tail, all [128, 4]-wide: the positive-logit exp for every group
            # runs as ONE tiny ACT op: ppos = exp(lpos/T + nbias), then
            # stot = sneg + ppos, row_loss = log(stot) - (lpos/T + nbias),
            # masked, then partition-reduce via ones-vector matmul.
            pprep = small.tile([128, N_GROUPS], FP32)
            nc.vector.scalar_tensor_tensor(
                out=pprep[:], in0=lpos[:], scalar=INV_T, in1=nbias[:],
                op0=mybir.AluOpType.mult, op1=mybir.AluOpType.add,
            )
            ppos = small.tile([128, N_GROUPS], FP32)
            nc.scalar.activation(out=ppos[:], in_=pprep[:], func=AF.Exp)
            stot = small.tile([128, N_GROUPS], FP32)
            nc.vector.tensor_add(stot[:], sneg[:], ppos[:])
            lt = small.tile([128, N_GROUPS], FP32)
            nc.scalar.activation(out=lt[:], in_=stot[:], func=AF.Ln)
            rloss = small.tile([128, N_GROUPS], FP32)
            nc.vector.tensor_sub(rloss[:], lt[:], pprep[:])
            mrl = small.tile([128, N_GROUPS], FP32)
            nc.vector.tensor_mul(mrl[:], rloss[:], cp[:, MSK_OFF:MSK_OFF + N_GROUPS])

            fps = psum_pool.tile([128, K], FP32, tag="gps")
            nc.tensor.matmul(
                out=fps[0:1, 0:N_GROUPS],
                lhsT=cp[:, ONE_OFF:ONE_OFF + 1],
                rhs=mrl[:, 0:N_GROUPS],
                start=True,
                stop=True,
                tile_position=(0, 0),
            )
            osb = small.tile([1, 1], FP32)
            nc.vector.reduce_sum(out=osb[0:1], in_=fps[0:1, 0:N_GROUPS], axis=AX.X)
            nc.sync.dma_start(out=out_h[:], in_=osb[:])

    return nc


def _pack_inputs(q, k, queue, cls_labels):
    """Host-side packing: per-core slab windows + padded per-class q/k rows."""
    import ml_dtypes

    in_maps = []
    for i in range(N_CORES):
        end = CLASS_ENDS[i]
        own_start = CLASS_ENDS[i - 1] if i > 0 else 0
        w0 = end - SLOTS  # slab window start (may include 1 unowned class)

        cpack = np.zeros((D, CPACK_W), dtype=np.float32)
        cpack[:, ONE_OFF] = 1.0
        qt = np.zeros((D, SLOTS * M_PAD), dtype=np.float32)

        for t in range(SLOTS):
            c = w0 + t
            if c < own_start:
                continue  # overlap slot: slab read but no rows assigned
            rows = np.nonzero(cls_labels == c)[0]
            if len(rows) > M_PAD:
                raise ValueError(
                    f"class {c} has {len(rows)} samples > M_PAD={M_PAD}"
                )
            g, s = divmod(t, 4)
            for j, n in enumerate(rows):
                p = 32 * s + j
                qt[:, M_PAD * t + j] = q[n]
                cpack[p, QR_OFF + g * D:QR_OFF + (g + 1) * D] = q[n]
                cpack[p, KR_OFF + g * D:KR_OFF + (g + 1) * D] = k[n]
                cpack[p, MSK_OFF + g] = 1.0

        slabs = np.ascontiguousarray(queue[w0:end], dtype=np.float32)
        if QDT == "bf16":
            slabs = slabs.astype(ml_dtypes.bfloat16)
            qt = qt.astype(ml_dtypes.bfloat16)

        in_maps.append({"cpack": cpack, "qt": qt, "slabs": slabs})
    return in_maps


def kernel(q, k, queue, class_weights, cls_labels):
    global last_run
    q = np.asarray(q, dtype=np.float32)
    k = np.asarray(k, dtype=np.float32)
    queue = np.asarray(queue, dtype=np.float32)
    cls_labels = np.asarray(cls_labels).astype(np.int64)

    in_maps = _pack_inputs(q, k, queue, cls_labels)
    nc = _build_nc()
    if not nc.is_finalized():
        nc.finalize()  # runs Bacc passes: reg alloc + event-semaphore wait split

    trace = bool(os.environ.get("BASS_TRACE"))
    res = bass_utils.run_bass_kernel_spmd(
        nc, in_maps, list(range(N_CORES)), trace=trace
    )
    last_run = res

    partial = sum(float(r["out"][0, 0]) for r in res.results)
    return np.float32(partial / N)



# revision 3
# speedup vs baseline: 1.6436x; 1.6436x over previous
"""Trainium2 Bass kernel for nn_ContrastByClassCalculator (MoCo-style
per-class-queue contrastive loss).

Math (reference):
    l_pos[n]  = q[n] . k[n]                                  # [N, 1]
    l_neg[n,:] = q[n] @ queue[cls_labels[n]]                 # [N, K]
    logits = concat([l_pos, l_neg], 1) / T                   # [N, 1+K]
    loss = mean_n( -log_softmax(logits)[n, 0] )

Design (v2): the queue [C=100, D=128, K=2048] dominates HBM traffic, so
the whole problem is DMA-bound.  The queue ships as fp8 (e3m4) to halve
traffic vs bf16.  The work is cut into 200 "class-halves" (class, 1024
k-columns), 25 per core -- a perfectly uniform shard.  On each core the
25 halves become 50 blocks of (class, 16 sample rows, 512 k-columns),
packed 8 blocks per PSUM bank ("bank-group", [128 rows, 512 cols]):
4 stripes of 32 partitions, 2 blocks per stripe via two accumulating
matmuls (zero-padded 32-col stationaries, start/stop accumulate).

Per bank-group the device computes only online-softmax statistics:
    mx = rowmax(psum)          (DVE)
    nb = -mx / T               (DVE tensor_scalar)
    S  = sum_k exp(l/T + nb)   (ACT exp with accum_out)
and DMAs out a [128, 14] stats tile (nb, S per bank-group).  Everything
else -- the positive logits, the cross-chunk log-sum-exp combine, the
final mean -- runs on the host in float64.  This removes the whole
serialized device tail (Ln, masking, partition reduction) of v1 and,
because no activation appears before the first DMA, the ACT table load
overlaps the slab DMA instead of blocking in the preamble.

Blocks are independent: each exports its own (nb, S), so the host
combine is exact regardless of how a class's 2048 columns are split
across blocks or cores.  Pad rows inside a stripe compute garbage stats
that the host simply never reads.
"""

import os

import numpy as np

import concourse.bacc as bacc
import concourse.mybir as mybir
import concourse.tile as tile
from concourse import bass_utils

# Problem constants (hardcoded per contract; kernel.py must be self-contained)
N = 512
D = 128
C = 100
K = 2048
T = 0.07
INV_T = float(1.0 / T)

N_CORES = 8
BLK_K = 512            # block width  = one PSUM bank
BLK_ROWS = 16          # block height = half a 32-row PE tile stripe
HALVES_PER_CORE = 25   # 200 class-halves / 8 cores
BLOCKS_PER_CORE = 50   # 2 blocks per half
BG_BLOCKS = 8          # blocks per bank-group (4 stripes x 2)
N_BG = 7               # ceil(50 / 8): 6 full groups + one 2-block group
SLAB_COLS = BLOCKS_PER_CORE * BLK_K   # 25600 fp8 bytes per partition
QT_COLS = BLOCKS_PER_CORE * 32        # one 32-col stationary per block

FP32 = mybir.dt.float32
FP8 = mybir.dt.float8e3
BF16 = mybir.dt.bfloat16

# stationary dtype: bf16 halves the q-quantization error of fp8 at
# negligible DMA cost (the stationaries are ~3 KB/partition total);
# BASS_QDT=f8 forces all-fp8.
QT_DT = BF16 if os.environ.get("BASS_QDT", "bf16") == "bf16" else FP8

# Results of the last hardware run (for test harnesses): BassKernelResults
last_run = None


def _bg_rows(g):
    nblk = min(BG_BLOCKS, BLOCKS_PER_CORE - g * BG_BLOCKS)
    return 32 * ((nblk + 1) // 2)


def _build_nc():
    """Single-core SPMD Bass/Tile program (identical on all 8 cores)."""
    nc = bacc.Bacc("TRN2")

    slabs_h = nc.dram_tensor("slabs", [D, SLAB_COLS], FP8, kind="ExternalInput")
    qt_h = nc.dram_tensor("qt", [D, QT_COLS], QT_DT, kind="ExternalInput")
    out_h = nc.dram_tensor("out", [D, 2 * N_BG], FP32, kind="ExternalOutput")

    AX = mybir.AxisListType
    AF = mybir.ActivationFunctionType

    with tile.TileContext(nc) as tc:
        with (
            tc.tile_pool(name="consts", bufs=1) as consts,
            tc.tile_pool(name="small", bufs=1) as small,
            tc.tile_pool(name="esc", bufs=2) as esc_pool,
            tc.tile_pool(name="psum", bufs=1, space="PSUM") as psum_pool,
        ):
            # Slab DMA: one chunk per bank-group (contiguous [128, 4096 B]
            # slices); first chunk then qt so matmuls can start early.
            slab = consts.tile([D, SLAB_COLS], FP8)
            nc.sync.dma_start(
                out=slab[:, 0:BG_BLOCKS * BLK_K],
                in_=slabs_h[:, 0:BG_BLOCKS * BLK_K],
            )
            qt = consts.tile([D, QT_COLS], QT_DT)
            nc.sync.dma_start(out=qt[:], in_=qt_h[:])
            for g in range(1, N_BG):
                c0 = g * BG_BLOCKS * BLK_K
                c1 = min(SLAB_COLS, (g + 1) * BG_BLOCKS * BLK_K)
                nc.sync.dma_start(out=slab[:, c0:c1], in_=slabs_h[:, c0:c1])

            # stats[p, 2g] = -rowmax/T, stats[p, 2g+1] = sum exp(l/T - rowmax/T)
            stats = small.tile([128, 2 * N_BG], FP32)
            nc.vector.memset(stats[:], 0.0)
            mx = small.tile([128, N_BG], FP32)

            for g in range(N_BG):
                nblk = min(BG_BLOCKS, BLOCKS_PER_CORE - g * BG_BLOCKS)
                rows = _bg_rows(g)
                ps = psum_pool.tile([128, BLK_K], FP32, tag=f"ps{g}")

                # 8 accumulating matmuls: per 32-row stripe, block A fills
                # rows 0-15 (start=True also zeroes 16-31), block B
                # accumulates rows 16-31.  Pairs are emitted adjacently so
                # only one PSUM accumulation group is ever pending per
                # bank; different stripes still overlap on the PE via
                # distinct 32-col column-groups.
                for u in range(nblk):
                    b = g * BG_BLOCKS + u
                    s, slot = divmod(u, 2)
                    nc.tensor.matmul(
                        out=ps[32 * s:32 * s + 32, :],
                        lhsT=qt[:, 32 * b:32 * (b + 1)],
                        rhs=slab[:, BLK_K * b:BLK_K * (b + 1)],
                        start=(slot == 0),
                        stop=(slot == 1) or (u == nblk - 1 and nblk % 2 == 1),
                        tile_position=(0, 32 * s),
                    )

                col = slice(g, g + 1)
                nc.vector.reduce_max(out=mx[0:rows, col], in_=ps[0:rows, :], axis=AX.X)
                nc.vector.tensor_scalar_mul(
                    out=stats[0:rows, 2 * g:2 * g + 1],
                    in0=mx[0:rows, col],
                    scalar1=-INV_T,
                )
                esc = esc_pool.tile([128, BLK_K], BF16, tag="esc")
                nc.scalar.activation(
                    out=esc[0:rows, :],
                    in_=ps[0:rows, :],
                    func=AF.Exp,
                    bias=stats[0:rows, 2 * g:2 * g + 1],
                    scale=INV_T,
                    accum_out=stats[0:rows, 2 * g + 1:2 * g + 2],
                )

            nc.sync.dma_start(out=out_h[:], in_=stats[:])

    return nc


def _pack_inputs(q, k, queue, cls_labels):
    """Host-side packing.

    Returns (in_maps, locs) where locs[n] is a list of (core, stats-col
    group, partition) triples covering sample n's 2048 negative columns.
    """
    import ml_dtypes

    cls_idx = [np.nonzero(cls_labels == c)[0] for c in range(C)]
    for c in range(C):
        if len(cls_idx[c]) > BLK_ROWS:
            raise ValueError(f"class {c} has {len(cls_idx[c])} > {BLK_ROWS} samples")

    halves = [(c, h) for c in range(C) for h in (0, 1)]
    locs = [[] for _ in range(N)]
    in_maps = []
    for i in range(N_CORES):
        mine = halves[HALVES_PER_CORE * i:HALVES_PER_CORE * (i + 1)]
        blocks = [(c, 1024 * h + BLK_K * j) for (c, h) in mine for j in (0, 1)]
        assert len(blocks) == BLOCKS_PER_CORE

        slab = np.empty((D, SLAB_COLS), dtype=ml_dtypes.float8_e3m4)
        qt = np.zeros((D, QT_COLS), dtype=np.float32)
        for b, (c, k0) in enumerate(blocks):
            slab[:, BLK_K * b:BLK_K * (b + 1)] = queue[c][:, k0:k0 + BLK_K]
            g, u = divmod(b, BG_BLOCKS)
            s, slot = divmod(u, 2)
            for j, n in enumerate(cls_idx[c]):
                qt[:, 32 * b + 16 * slot + j] = q[n]
                locs[n].append((i, g, 32 * s + 16 * slot + j))

        np_qt_dt = ml_dtypes.bfloat16 if QT_DT == BF16 else ml_dtypes.float8_e3m4
        in_maps.append({"slabs": slab, "qt": qt.astype(np_qt_dt)})
    return in_maps, locs


def _combine(stats_list, locs, lpos_scaled):
    """Float64 host-side log-sum-exp combine of per-block stats."""
    total = 0.0
    for n in range(N):
        M = np.array([-float(stats_list[i][p, 2 * g]) for (i, g, p) in locs[n]])
        S = np.array([float(stats_list[i][p, 2 * g + 1]) for (i, g, p) in locs[n]])
        lp = lpos_scaled[n]
        B = max(lp, M.max())
        tot = np.exp(lp - B) + (S * np.exp(M - B)).sum()
        total += B + np.log(tot) - lp
    return total / N


def kernel(q, k, queue, class_weights, cls_labels):
    global last_run
    q = np.asarray(q, dtype=np.float32)
    k = np.asarray(k, dtype=np.float32)
    queue = np.asarray(queue, dtype=np.float32)
    cls_labels = np.asarray(cls_labels).astype(np.int64)

    in_maps, locs = _pack_inputs(q, k, queue, cls_labels)
    nc = _build_nc()
    if not nc.is_finalized():
        nc.finalize()

    trace = bool(os.environ.get("BASS_TRACE"))
    res = bass_utils.run_bass_kernel_spmd(
        nc, in_maps, list(range(N_CORES)), trace=trace
    )
    last_run = res

    stats_list = [np.asarray(r["out"], dtype=np.float64) for r in res.results]
    lpos_scaled = (q.astype(np.float64) * k.astype(np.float64)).sum(1) / T
    return np.float32(_combine(stats_list, locs, lpos_scaled))


# revision 5
# speedup vs baseline: 1.6457x; 1.0013x over previous
"""Trainium2 Bass kernel for nn_ContrastByClassCalculator (MoCo-style
per-class-queue contrastive loss).

Math (reference):
    l_pos[n]  = q[n] . k[n]                                  # [N, 1]
    l_neg[n,:] = q[n] @ queue[cls_labels[n]]                 # [N, K]
    logits = concat([l_pos, l_neg], 1) / T                   # [N, 1+K]
    loss = mean_n( -log_softmax(logits)[n, 0] )

Design (v2): the queue [C=100, D=128, K=2048] dominates HBM traffic, so
the whole problem is DMA-bound.  The queue ships as fp8 (e3m4) to halve
traffic vs bf16.  The work is cut into 200 "class-halves" (class, 1024
k-columns), 25 per core -- a perfectly uniform shard.  On each core the
25 halves become 50 blocks of (class, 16 sample rows, 512 k-columns),
packed 8 blocks per PSUM bank ("bank-group", [128 rows, 512 cols]):
4 stripes of 32 partitions, 2 blocks per stripe via two accumulating
matmuls (zero-padded 32-col stationaries, start/stop accumulate).

Per bank-group the device computes only online-softmax statistics:
    mx = rowmax(psum)          (DVE)
    nb = -mx / T               (DVE tensor_scalar)
    S  = sum_k exp(l/T + nb)   (ACT exp with accum_out)
and DMAs out a [128, 14] stats tile (nb, S per bank-group).  Everything
else -- the positive logits, the cross-chunk log-sum-exp combine, the
final mean -- runs on the host in float64.  This removes the whole
serialized device tail (Ln, masking, partition reduction) of v1 and,
because no activation appears before the first DMA, the ACT table load
overlaps the slab DMA instead of blocking in the preamble.

Blocks are independent: each exports its own (nb, S), so the host
combine is exact regardless of how a class's 2048 columns are split
across blocks or cores.  Pad rows inside a stripe compute garbage stats
that the host simply never reads.
"""

import os

import numpy as np

import concourse.bacc as bacc
import concourse.mybir as mybir
import concourse.tile as tile
from concourse import bass_utils

# Problem constants (hardcoded per contract; kernel.py must be self-contained)
N = 512
D = 128
C = 100
K = 2048
T = 0.07
INV_T = float(1.0 / T)

N_CORES = 8
BLK_K = 512            # block width  = one PSUM bank
BLK_ROWS = 16          # block height = half a 32-row PE tile stripe
HALVES_PER_CORE = 25   # 200 class-halves / 8 cores
BLOCKS_PER_CORE = 50   # 2 blocks per half
BG_BLOCKS = 8          # blocks per bank-group (4 stripes x 2)
N_BG = 7               # ceil(50 / 8): 6 full groups + one 2-block group
SLAB_COLS = BLOCKS_PER_CORE * BLK_K   # 25600 fp8 bytes per partition
QT_COLS = BLOCKS_PER_CORE * 32        # one 32-col stationary per block

FP32 = mybir.dt.float32
FP8 = mybir.dt.float8e3
BF16 = mybir.dt.bfloat16

# stationary dtype: fp8 (e3m4) keeps the qt DMA small (0.2 MB) so the
# first matmuls can start early; total loss error stays ~7e-4.
# BASS_QDT=bf16 halves the q-quantization error at 2x the qt traffic.
QT_DT = BF16 if os.environ.get("BASS_QDT", "f8") == "bf16" else FP8

# Results of the last hardware run (for test harnesses): BassKernelResults
last_run = None


def _bg_rows(g):
    nblk = min(BG_BLOCKS, BLOCKS_PER_CORE - g * BG_BLOCKS)
    return 32 * ((nblk + 1) // 2)


def _build_nc():
    """Single-core SPMD Bass/Tile program (identical on all 8 cores)."""
    nc = bacc.Bacc("TRN2")

    slabs_h = nc.dram_tensor("slabs", [D, SLAB_COLS], FP8, kind="ExternalInput")
    qt_h = nc.dram_tensor("qt", [D, QT_COLS], QT_DT, kind="ExternalInput")
    out_h = nc.dram_tensor("out", [D, 2 * N_BG], FP32, kind="ExternalOutput")

    AX = mybir.AxisListType
    AF = mybir.ActivationFunctionType

    with tile.TileContext(nc) as tc:
        with (
            tc.tile_pool(name="consts", bufs=1) as consts,
            tc.tile_pool(name="small", bufs=1) as small,
            tc.tile_pool(name="esc", bufs=2) as esc_pool,
            tc.tile_pool(name="psum", bufs=1, space="PSUM") as psum_pool,
        ):
            # Slab DMA: one chunk per bank-group (contiguous [128, 4096 B]
            # slices).  Each HWDGE dispatch costs ~0.7 us serialized on its
            # issuing sequencer, so the 8 input DMAs alternate between the
            # two physical HWDGE rings (Sync and Scalar sequencers) to halve
            # the dispatch head latency.  qt leads its ring: every matmul
            # needs it.
            slab = consts.tile([D, SLAB_COLS], FP8)
            qt = consts.tile([D, QT_COLS], QT_DT)
            nc.sync.dma_start(out=qt[:], in_=qt_h[:])
            for g in range(N_BG):
                c0 = g * BG_BLOCKS * BLK_K
                c1 = min(SLAB_COLS, (g + 1) * BG_BLOCKS * BLK_K)
                eng = nc.scalar if g % 2 == 0 else nc.sync
                eng.dma_start(out=slab[:, c0:c1], in_=slabs_h[:, c0:c1])

            # stats[p, 2g] = -rowmax/T, stats[p, 2g+1] = sum exp(l/T - rowmax/T)
            stats = small.tile([128, 2 * N_BG], FP32)
            nc.vector.memset(stats[:], 0.0)
            mx = small.tile([128, N_BG], FP32)

            # Warm matmul into the spare 8th PSUM bank: PE drops to a low
            # p-state after long idle gaps (HAM); a tiny matmul right after
            # the DMA dispatches keeps the idle gap before the first real
            # matmul under the ~3.4 us throttle threshold.
            warm = small.tile([128, 32], QT_DT)
            nc.vector.memset(warm[:], 0.0)
            wps = psum_pool.tile([32, 32], FP32, tag="warm")
            nc.tensor.matmul(
                out=wps[:], lhsT=warm[:], rhs=warm[:], start=True, stop=True
            )

            for g in range(N_BG):
                nblk = min(BG_BLOCKS, BLOCKS_PER_CORE - g * BG_BLOCKS)
                rows = _bg_rows(g)
                ps = psum_pool.tile([128, BLK_K], FP32, tag=f"ps{g}")

                # 8 accumulating matmuls: per 32-row stripe, block A fills
                # rows 0-15 (start=True also zeroes 16-31), block B
                # accumulates rows 16-31.  Pairs are emitted adjacently so
                # only one PSUM accumulation group is ever pending per
                # bank; different stripes still overlap on the PE via
                # distinct 32-col column-groups.
                for u in range(nblk):
                    b = g * BG_BLOCKS + u
                    s, slot = divmod(u, 2)
                    nc.tensor.matmul(
                        out=ps[32 * s:32 * s + 32, :],
                        lhsT=qt[:, 32 * b:32 * (b + 1)],
                        rhs=slab[:, BLK_K * b:BLK_K * (b + 1)],
                        start=(slot == 0),
                        stop=(slot == 1) or (u == nblk - 1 and nblk % 2 == 1),
                        tile_position=(0, 32 * s),
                    )

                col = slice(g, g + 1)
                nc.vector.reduce_max(out=mx[0:rows, col], in_=ps[0:rows, :], axis=AX.X)
                nc.vector.tensor_scalar_mul(
                    out=stats[0:rows, 2 * g:2 * g + 1],
                    in0=mx[0:rows, col],
                    scalar1=-INV_T,
                )
                esc = esc_pool.tile([128, BLK_K], BF16, tag="esc")
                nc.scalar.activation(
                    out=esc[0:rows, :],
                    in_=ps[0:rows, :],
                    func=AF.Exp,
                    bias=stats[0:rows, 2 * g:2 * g + 1],
                    scale=INV_T,
                    accum_out=stats[0:rows, 2 * g + 1:2 * g + 2],
                )

            nc.sync.dma_start(out=out_h[:], in_=stats[:])

    return nc


def _pack_inputs(q, k, queue, cls_labels):
    """Host-side packing.

    Returns (in_maps, locs) where locs[n] is a list of (core, stats-col
    group, partition) triples covering sample n's 2048 negative columns.
    """
    import ml_dtypes

    cls_idx = [np.nonzero(cls_labels == c)[0] for c in range(C)]
    for c in range(C):
        if len(cls_idx[c]) > BLK_ROWS:
            raise ValueError(f"class {c} has {len(cls_idx[c])} > {BLK_ROWS} samples")

    halves = [(c, h) for c in range(C) for h in (0, 1)]
    locs = [[] for _ in range(N)]
    in_maps = []
    for i in range(N_CORES):
        mine = halves[HALVES_PER_CORE * i:HALVES_PER_CORE * (i + 1)]
        blocks = [(c, 1024 * h + BLK_K * j) for (c, h) in mine for j in (0, 1)]
        assert len(blocks) == BLOCKS_PER_CORE

        slab = np.empty((D, SLAB_COLS), dtype=ml_dtypes.float8_e3m4)
        qt = np.zeros((D, QT_COLS), dtype=np.float32)
        for b, (c, k0) in enumerate(blocks):
            slab[:, BLK_K * b:BLK_K * (b + 1)] = queue[c][:, k0:k0 + BLK_K]
            g, u = divmod(b, BG_BLOCKS)
            s, slot = divmod(u, 2)
            for j, n in enumerate(cls_idx[c]):
                qt[:, 32 * b + 16 * slot + j] = q[n]
                locs[n].append((i, g, 32 * s + 16 * slot + j))

        np_qt_dt = ml_dtypes.bfloat16 if QT_DT == BF16 else ml_dtypes.float8_e3m4
        in_maps.append({"slabs": slab, "qt": qt.astype(np_qt_dt)})
    return in_maps, locs


def _combine(stats_list, locs, lpos_scaled):
    """Float64 host-side log-sum-exp combine of per-block stats."""
    total = 0.0
    for n in range(N):
        M = np.array([-float(stats_list[i][p, 2 * g]) for (i, g, p) in locs[n]])
        S = np.array([float(stats_list[i][p, 2 * g + 1]) for (i, g, p) in locs[n]])
        lp = lpos_scaled[n]
        B = max(lp, M.max())
        tot = np.exp(lp - B) + (S * np.exp(M - B)).sum()
        total += B + np.log(tot) - lp
    return total / N


def kernel(q, k, queue, class_weights, cls_labels):
    global last_run
    q = np.asarray(q, dtype=np.float32)
    k = np.asarray(k, dtype=np.float32)
    queue = np.asarray(queue, dtype=np.float32)
    cls_labels = np.asarray(cls_labels).astype(np.int64)

    in_maps, locs = _pack_inputs(q, k, queue, cls_labels)
    nc = _build_nc()
    if not nc.is_finalized():
        nc.finalize()

    trace = bool(os.environ.get("BASS_TRACE"))
    res = bass_utils.run_bass_kernel_spmd(
        nc, in_maps, list(range(N_CORES)), trace=trace
    )
    last_run = res

    stats_list = [np.asarray(r["out"], dtype=np.float64) for r in res.results]
    lpos_scaled = (q.astype(np.float64) * k.astype(np.float64)).sum(1) / T
    return np.float32(_combine(stats_list, locs, lpos_scaled))


# revision 7
# speedup vs baseline: 1.6986x; 1.0321x over previous
"""Trainium2 Bass kernel for nn_ContrastByClassCalculator (MoCo-style
per-class-queue contrastive loss).

Math (reference):
    l_pos[n]  = q[n] . k[n]                                  # [N, 1]
    l_neg[n,:] = q[n] @ queue[cls_labels[n]]                 # [N, K]
    logits = concat([l_pos, l_neg], 1) / T                   # [N, 1+K]
    loss = mean_n( -log_softmax(logits)[n, 0] )

Design (v3): the queue [C=100, D=128, K=2048] dominates HBM traffic, so
the whole problem is DMA-bound.  The queue ships as fp8 (e3m4) to halve
traffic vs bf16.  The work is cut into 200 "class-halves" (class, 1024
k-columns), 25 per core -- a perfectly uniform shard.  On each core the
25 halves become 50 blocks of (class, 16 sample rows, 512 k-columns).
Blocks 0-47 pack 8 per PSUM bank ("bank-group", [128 rows, 512 cols]):
4 stripes of 32 partitions, 2 blocks per stripe via two accumulating
matmuls (zero-padded 32-col stationaries, start/stop accumulate).  The
final two blocks split column-wise into the last two bank-groups of
[32 rows, 256 cols] so the end-of-pipeline max+exp chain is half-width.

Per bank-group the device computes only online-softmax statistics:
    mx = rowmax(psum)          (DVE)
    nb = -mx / T               (DVE tensor_scalar)
    S  = sum_k exp(l/T + nb)   (ACT exp with accum_out)
and DMAs out a [128, 16] stats tile (nb, S per bank-group; the bulk
ships right after bank-group 5, a tiny 16-byte-per-partition finisher
after the last exp).  Everything else -- the positive logits, the
cross-chunk log-sum-exp combine, the final mean -- runs on the host in
float64.  Blocks are independent: each exports its own (nb, S), so the
host combine is exact regardless of how a class's 2048 columns are
split across blocks or cores.  Pad rows inside a stripe compute garbage
stats that the host simply never reads.

Scheduling notes: the 8 input DMAs alternate across the two physical
HWDGE rings (Sync / Scalar sequencers) because each dispatch costs
~0.7 us serialized on its ring; no activation precedes the first DMA so
the ACT table load overlaps the slab stream; and a tiny "warm" matmul
precedes every bank-group so PE idle gaps stay under the ~3.4 us HAM
p-state throttle threshold while waiting for chunk DMAs.
"""

import os

import numpy as np

import concourse.bacc as bacc
import concourse.mybir as mybir
import concourse.tile as tile
from concourse import bass_utils

# Problem constants (hardcoded per contract; kernel.py must be self-contained)
N = 512
D = 128
C = 100
K = 2048
T = 0.07
INV_T = float(1.0 / T)

N_CORES = 8
BLK_K = 512            # block width  = one PSUM bank
BLK_ROWS = 16          # block height = half a 32-row PE tile stripe
HALVES_PER_CORE = 25   # 200 class-halves / 8 cores
BLOCKS_PER_CORE = 50   # 2 blocks per half
BG_BLOCKS = 8          # blocks per full bank-group (4 stripes x 2)
N_FULL_BG = 6          # blocks 0-47
N_BG = 8               # + two [32, 256] tail groups from blocks 48/49
TAIL_W = 256
SLAB_COLS = BLOCKS_PER_CORE * BLK_K   # 25600 fp8 bytes per partition
QT_COLS = BLOCKS_PER_CORE * 32        # one 32-col stationary per block

FP32 = mybir.dt.float32
FP8 = mybir.dt.float8e3
BF16 = mybir.dt.bfloat16

# stationary dtype: fp8 (e3m4) keeps the qt DMA small (0.2 MB) so the
# first matmuls can start early; total loss error stays ~7e-4.
# BASS_QDT=bf16 halves the q-quantization error at 2x the qt traffic.
QT_DT = BF16 if os.environ.get("BASS_QDT", "f8") == "bf16" else FP8

# Results of the last hardware run (for test harnesses): BassKernelResults
last_run = None


def _build_nc():
    """Single-core SPMD Bass/Tile program (identical on all 8 cores)."""
    nc = bacc.Bacc("TRN2")

    slabs_h = nc.dram_tensor("slabs", [D, SLAB_COLS], FP8, kind="ExternalInput")
    qt_h = nc.dram_tensor("qt", [D, QT_COLS], QT_DT, kind="ExternalInput")
    out_h = nc.dram_tensor("out", [D, 2 * N_BG], FP32, kind="ExternalOutput")

    AX = mybir.AxisListType
    AF = mybir.ActivationFunctionType

    with tile.TileContext(nc) as tc:
        with (
            tc.tile_pool(name="consts", bufs=1) as consts,
            tc.tile_pool(name="small", bufs=1) as small,
            tc.tile_pool(name="esc", bufs=2) as esc_pool,
            tc.tile_pool(name="psum", bufs=1, space="PSUM") as psum_pool,
        ):
            # Input DMAs: qt leads its ring (every matmul needs it); one
            # slab chunk per full bank-group plus one covering both tail
            # groups, alternating rings.
            slab = consts.tile([D, SLAB_COLS], FP8)
            qt = consts.tile([D, QT_COLS], QT_DT)
            nc.sync.dma_start(out=qt[:], in_=qt_h[:])
            bounds = [g * BG_BLOCKS * BLK_K for g in range(N_FULL_BG)]
            bounds += [N_FULL_BG * BG_BLOCKS * BLK_K, SLAB_COLS]
            for ci in range(len(bounds) - 1):
                c0, c1 = bounds[ci], bounds[ci + 1]
                eng = nc.scalar if ci % 2 == 0 else nc.sync
                eng.dma_start(out=slab[:, c0:c1], in_=slabs_h[:, c0:c1])

            # stats[p, 2g] = -rowmax/T, stats[p, 2g+1] = sum exp(l/T - rowmax/T)
            stats = small.tile([128, 2 * N_BG], FP32)
            nc.vector.memset(stats[:], 0.0)
            mx = small.tile([128, N_BG], FP32)
            warm = small.tile([128, 32], QT_DT)
            nc.vector.memset(warm[:], 0.0)

            ps_tiles = [
                psum_pool.tile([128, BLK_K], FP32, tag=f"ps{g}", name=f"ps{g}")
                for g in range(N_FULL_BG)
            ] + [
                psum_pool.tile([32, TAIL_W], FP32, tag=f"ps{g}", name=f"ps{g}")
                for g in range(N_FULL_BG, N_BG)
            ]

            def softmax_stats(g, rows, w):
                col = slice(g, g + 1)
                ps = ps_tiles[g]
                nc.vector.reduce_max(
                    out=mx[0:rows, col], in_=ps[0:rows, 0:w], axis=AX.X
                )
                nc.vector.tensor_scalar_mul(
                    out=stats[0:rows, 2 * g:2 * g + 1],
                    in0=mx[0:rows, col],
                    scalar1=-INV_T,
                )
                esc = esc_pool.tile([128, BLK_K], BF16, tag="esc")
                nc.scalar.activation(
                    out=esc[0:rows, 0:w],
                    in_=ps[0:rows, 0:w],
                    func=AF.Exp,
                    bias=stats[0:rows, 2 * g:2 * g + 1],
                    scale=INV_T,
                    accum_out=stats[0:rows, 2 * g + 1:2 * g + 2],
                )

            for g in range(N_FULL_BG):
                # Tiny warm matmul into the (re-zeroed-later) tail bank: runs
                # during any data stall ahead of this group's matmuls,
                # keeping the PE p-state up.  Its junk is erased by the tail
                # group's start=True matmul.
                nc.tensor.matmul(
                    out=ps_tiles[N_BG - 1][0:32, 0:32],
                    lhsT=warm[:],
                    rhs=warm[:],
                    start=True,
                    stop=True,
                )
                ps = ps_tiles[g]
                for u in range(BG_BLOCKS):
                    b = g * BG_BLOCKS + u
                    s, slot = divmod(u, 2)
                    nc.tensor.matmul(
                        out=ps[32 * s:32 * s + 32, :],
                        lhsT=qt[:, 32 * b:32 * (b + 1)],
                        rhs=slab[:, BLK_K * b:BLK_K * (b + 1)],
                        start=(slot == 0),
                        stop=(slot == 1),
                        tile_position=(0, 32 * s),
                    )
                softmax_stats(g, 128, BLK_K)
                if g == N_FULL_BG - 1:
                    # bulk of the stats ships early; only the tail groups'
                    # 16 B/partition ride the final DMA.
                    nc.sync.dma_start(
                        out=out_h[:, 0:2 * N_FULL_BG],
                        in_=stats[:, 0:2 * N_FULL_BG],
                    )

            # Tail: blocks 48/49 (the odd class-half) as two [32, 256]
            # groups; sub-block columns are packed a0|a1|b0|b1 so each
            # group's rhs slices stay contiguous.
            tc0 = N_FULL_BG * BG_BLOCKS * BLK_K
            for t, g in enumerate(range(N_FULL_BG, N_BG)):
                ps = ps_tiles[g]
                for slot in (0, 1):
                    b = 48 + slot
                    c0 = tc0 + (2 * t + slot) * TAIL_W
                    nc.tensor.matmul(
                        out=ps[0:32, :],
                        lhsT=qt[:, 32 * b:32 * (b + 1)],
                        rhs=slab[:, c0:c0 + TAIL_W],
                        start=(slot == 0),
                        stop=(slot == 1),
                        tile_position=(0, 0),
                    )
                softmax_stats(g, 32, TAIL_W)

            nc.sync.dma_start(
                out=out_h[:, 2 * N_FULL_BG:], in_=stats[:, 2 * N_FULL_BG:]
            )

    return nc


def _pack_inputs(q, k, queue, cls_labels):
    """Host-side packing.

    Returns (in_maps, locs) where locs[n] is a list of (core, stats-col
    group, partition) triples covering sample n's 2048 negative columns.
    """
    import ml_dtypes

    cls_idx = [np.nonzero(cls_labels == c)[0] for c in range(C)]
    for c in range(C):
        if len(cls_idx[c]) > BLK_ROWS:
            raise ValueError(f"class {c} has {len(cls_idx[c])} > {BLK_ROWS} samples")

    halves = [(c, h) for c in range(C) for h in (0, 1)]
    locs = [[] for _ in range(N)]
    in_maps = []
    np_qt_dt = ml_dtypes.bfloat16 if QT_DT == BF16 else ml_dtypes.float8_e3m4
    for i in range(N_CORES):
        mine = halves[HALVES_PER_CORE * i:HALVES_PER_CORE * (i + 1)]
        blocks = [(c, 1024 * h + BLK_K * j) for (c, h) in mine for j in (0, 1)]
        assert len(blocks) == BLOCKS_PER_CORE

        slab = np.empty((D, SLAB_COLS), dtype=ml_dtypes.float8_e3m4)
        qt = np.zeros((D, QT_COLS), dtype=np.float32)
        for b, (c, k0) in enumerate(blocks[:48]):
            slab[:, BLK_K * b:BLK_K * (b + 1)] = queue[c][:, k0:k0 + BLK_K]
            g, u = divmod(b, BG_BLOCKS)
            s, slot = divmod(u, 2)
            for j, n in enumerate(cls_idx[c]):
                qt[:, 32 * b + 16 * slot + j] = q[n]
                locs[n].append((i, g, 32 * s + 16 * slot + j))

        # tail blocks 48/49 -> four 256-col sub-blocks in groups 6/7,
        # column layout [48a|49a|48b|49b]
        tc0 = 48 * BLK_K
        for slot, (c, k0) in enumerate(blocks[48:]):
            for t in (0, 1):
                c0 = tc0 + (2 * t + slot) * TAIL_W
                slab[:, c0:c0 + TAIL_W] = (
                    queue[c][:, k0 + t * TAIL_W:k0 + (t + 1) * TAIL_W]
                )
            b = 48 + slot
            for j, n in enumerate(cls_idx[c]):
                qt[:, 32 * b + 16 * slot + j] = q[n]
                locs[n].append((i, 6, 16 * slot + j))
                locs[n].append((i, 7, 16 * slot + j))

        in_maps.append({"slabs": slab, "qt": qt.astype(np_qt_dt)})
    return in_maps, locs


def _combine(stats_list, locs, lpos_scaled):
    """Float64 host-side log-sum-exp combine of per-block stats."""
    total = 0.0
    for n in range(N):
        M = np.array([-float(stats_list[i][p, 2 * g]) for (i, g, p) in locs[n]])
        S = np.array([float(stats_list[i][p, 2 * g + 1]) for (i, g, p) in locs[n]])
        lp = lpos_scaled[n]
        B = max(lp, M.max())
        tot = np.exp(lp - B) + (S * np.exp(M - B)).sum()
        total += B + np.log(tot) - lp
    return total / N


def kernel(q, k, queue, class_weights, cls_labels):
    global last_run
    q = np.asarray(q, dtype=np.float32)
    k = np.asarray(k, dtype=np.float32)
    queue = np.asarray(queue, dtype=np.float32)
    cls_labels = np.asarray(cls_labels).astype(np.int64)

    in_maps, locs = _pack_inputs(q, k, queue, cls_labels)
    nc = _build_nc()
    if not nc.is_finalized():
        nc.finalize()

    trace = bool(os.environ.get("BASS_TRACE"))
    res = bass_utils.run_bass_kernel_spmd(
        nc, in_maps, list(range(N_CORES)), trace=trace
    )
    last_run = res

    stats_list = [np.asarray(r["out"], dtype=np.float64) for r in res.results]
    lpos_scaled = (q.astype(np.float64) * k.astype(np.float64)).sum(1) / T
    return np.float32(_combine(stats_list, locs, lpos_scaled))
